# revision 14
# baseline (speedup 1.0000x reference)
"""Trainium2 Bass kernel for nn_BottleneckSparse2D (submanifold sparse bottleneck
block, gnn_message_passing).

Strategy (8 NeuronCores, SPMD, sites sharded):
  N=260000 active sites are sharded as contiguous slabs of 32500 sites/core,
  zero-padded to 32768. The rulebook gather (halo gather) is applied on the
  host to the *input features* (gather commutes with the 1x1 conv + BN + relu),
  so each core receives dense, GEMM-ready, transposed per-offset feature
  blocks. BN batch statistics are reduced across cores on the host between
  launches (sums / second-moment matrices only; tiny tensors).

  L1: per-core feature moments  [sum(x), x^T x]      (for BN1 + BN_s stats)
  L2: z = g_k @ W1' -> relu(z+b1) -> sum_k Wk^T h_k  (the 3x3 subm conv),
      emits out2_raw^T slab + BN2 partial sums. BN1 scale is folded into W1.
  L3: hhat = relu(out2_raw + b2/a2) -> moments       (for BN3; a2 handled on host)
  L4: out^T = relu(W3''^T hhat^T + Ws'^T feat^T + beta)

  Invalid rulebook entries (and padded sites) gather a synthetic feature row
  x* chosen so that relu(x*@W1' + b1) == 0 exactly (with margin), so they
  contribute nothing to the conv and keep padded sites' outputs at exact 0
  (which keeps the cross-core BN2 sums clean).

L2 matmuls run in bf16 by default (BASS_L2_DT=float32r to fall back);
L3/L4 matmuls in float32r. Stats are accumulated in fp32/PSUM throughout.
"""

import os
import numpy as np
import ml_dtypes

import concourse.bacc as bacc
import concourse.tile as tile
from concourse import bass, mybir
from concourse.bass_utils import run_bass_kernel_spmd
from concourse.masks import make_identity

F32 = mybir.dt.float32
L2DT = getattr(mybir.dt, os.environ.get("BASS_L2_DT", "bfloat16"))
FR = getattr(mybir.dt, os.environ.get("BASS_L34_DT", "bfloat16"))
FR_NP = None  # set below
L2DT_NP = mybir.dt.np(L2DT)
FR_NP = mybir.dt.np(FR)

N = 260000
CORES = 8
NSLAB = N // CORES            # 32500
NPAD = 32768                  # per-core padded slab
TS = 512                      # PE site-tile
DTS = 1024                    # DMA site-tile
NDT = NPAD // DTS             # 32
CIN = 64
CMID = 64
COUT = 256
K9 = 9
NBLK = 5                      # 4 offset-pairs + 1 solo block
BN_EPS = 1e-5
MARGIN = 4.0

TRACE = bool(int(os.environ.get("BASS_KERNEL_TRACE", "0")))
LAST_EXEC_NS = {}
LAST_IN_MAPS = {}

_BUILT = {}

RELU = mybir.ActivationFunctionType.Relu


def _run(name, nc, in_maps):
    if TRACE:
        LAST_IN_MAPS[name] = in_maps
    res = run_bass_kernel_spmd(nc, in_maps, core_ids=list(range(CORES)))
    LAST_EXEC_NS[name] = res.exec_time_ns
    return res.results


# ---------------------------------------------------------------- L1: moments
def build_l1(repeat=1):
    nc = bacc.Bacc()
    feat = nc.declare_dram_parameter("feat", [NPAD, CIN], F32, isOutput=False)
    mom = nc.declare_dram_parameter("mom", [128, CIN], F32, isOutput=True)
    with tile.TileContext(nc) as tc:
        with tc.tile_pool(name="sb", bufs=2) as sb, \
             tc.tile_pool(name="ps", bufs=1, space="PSUM") as ps, \
             tc.tile_pool(name="osb", bufs=1) as osb:
            acc0 = ps.tile([128, CIN], F32, tag="acc0")  # col-group 0 (rows 0:64)
            acc1 = ps.tile([128, CIN], F32, tag="acc1")  # col-group 1 (rows 64:128)
            # contiguous per-partition layout: partition p holds sites
            # [p*256, (p+1)*256); chunk j covers t in [j*64, (j+1)*64)
            feat_r = feat[:].rearrange("(p t) c -> p t c", p=128)  # [128, 256, 64]
            nchunk = 4
            tper = 256 // nchunk
            n_mm = nchunk * tper * repeat
            i = 0
            for j in [jj for _ in range(repeat) for jj in range(nchunk)]:
                ck = sb.tile([128, tper, CIN], F32, tag="ck")
                nc.sync.dma_start(out=ck[:], in_=feat_r[:, j * tper:(j + 1) * tper, :])
                for t in range(tper):
                    half = t % 2
                    acc = acc0 if half == 0 else acc1
                    nc.tensor.matmul(
                        out=acc[half * CIN:(half + 1) * CIN, :],
                        lhsT=ck[:, t, :],
                        rhs=ck[:, t, :],
                        tile_position=(0, half * CIN),
                        start=(i <= 1), stop=(i >= n_mm - 2),
                    )
                    i += 1
            res = osb.tile([128, CIN], F32)
            nc.scalar.copy(out=res[0:CIN, :], in_=acc0[0:CIN, :])
            nc.scalar.copy(out=res[CIN:128, :], in_=acc1[CIN:128, :])
            nc.sync.dma_start(out=mom[:], in_=res[:])
    nc.compile()
    return nc


# ------------------------------------------------------- L2: submanifold conv
def build_l2(repeat=1):
    nc = bacc.Bacc()
    gf = nc.declare_dram_parameter("gf", [NBLK, 128, NPAD], L2DT, isOutput=False)
    wbd = nc.declare_dram_parameter("wbd", [128, 128], L2DT, isOutput=False)
    wkp = nc.declare_dram_parameter("wkp", [NBLK, 128, CMID], L2DT, isOutput=False)
    b1p = nc.declare_dram_parameter("b1p", [128, 1], F32, isOutput=False)
    o2t = nc.declare_dram_parameter("o2t", [128, NPAD // 2], L2DT, isOutput=True)
    with tile.TileContext(nc) as tc:
        with tc.tile_pool(name="wsb", bufs=1) as wsb, \
             tc.tile_pool(name="gsb", bufs=3) as gsb, \
             tc.tile_pool(name="hsb", bufs=5) as hsb, \
             tc.tile_pool(name="zps", bufs=3, space="PSUM") as zps, \
             tc.tile_pool(name="ops", bufs=2, space="PSUM") as ops, \
             tc.tile_pool(name="osb", bufs=4) as osb:
            wbd_t = wsb.tile([128, 128], L2DT, tag="wbd")
            nc.sync.dma_start(out=wbd_t[:], in_=wbd[:])
            wkp_t = wsb.tile([128, NBLK, CMID], L2DT, tag="wkp")
            nc.sync.dma_start(out=wkp_t[:], in_=wkp[:].rearrange("b p c -> p b c"))
            b1t = wsb.tile([128, 1], F32, tag="b1t")
            nc.sync.dma_start(out=b1t[:], in_=b1p[:])
            for d in [dd for _ in range(repeat) for dd in range(NDT)]:
                sl = slice(d * DTS, (d + 1) * DTS)
                # out2 for this 1024-site tile: partitions 0:64 = sites
                # [d*1024, +512) ; partitions 64:128 = sites [d*1024+512, +512)
                o = ops.tile([128, TS], F32, tag="o")  # 1 bank
                for b in range(NBLK):
                    gt = gsb.tile([128, DTS], L2DT, tag=f"g{b}")
                    nc.sync.dma_start(out=gt[:], in_=gf[b, :, sl])
                    z = zps.tile([128, DTS], F32, tag="z")  # 2 banks
                    nc.tensor.matmul(out=z[:, 0:TS], lhsT=wbd_t[:],
                                     rhs=gt[:, 0:TS], start=True, stop=True)
                    nc.tensor.matmul(out=z[:, TS:DTS], lhsT=wbd_t[:],
                                     rhs=gt[:, TS:DTS], start=True, stop=True)
                    h = hsb.tile([128, DTS], L2DT, tag="h")
                    if b >= 3:
                        nc.vector.tensor_scalar(
                            out=h[:], in0=z[:], scalar1=b1t[:], scalar2=0.0,
                            op0=mybir.AluOpType.add, op1=mybir.AluOpType.max)
                    else:
                        # relu(z + b1) on ACT (scale pre-folded into W1)
                        nc.scalar.activation(out=h[:], in_=z[:], func=RELU,
                                             bias=b1t[:], scale=1.0)
                    # two concurrent M=64 matmuls in PE column groups 0/1
                    nc.tensor.matmul(out=o[0:CMID, :], lhsT=wkp_t[:, b, :],
                                     rhs=h[:, 0:TS], tile_position=(0, 0),
                                     start=(b == 0), stop=(b == NBLK - 1))
                    nc.tensor.matmul(out=o[CMID:128, :], lhsT=wkp_t[:, b, :],
                                     rhs=h[:, TS:DTS], tile_position=(0, 64),
                                     start=(b == 0), stop=(b == NBLK - 1))
                ot = osb.tile([128, TS], L2DT, tag="ot")
                nc.vector.tensor_copy(out=ot[:], in_=o[:])
                nc.sync.dma_start(out=o2t[:, d * TS:(d + 1) * TS], in_=ot[:])
    nc.compile()
    return nc


# ------------------------------------------------------------- L3: h2 moments
def build_l3(repeat=1):
    nc = bacc.Bacc()
    oft = nc.declare_dram_parameter("oft", [128, NPAD], FR, isOutput=False)
    b2p = nc.declare_dram_parameter("b2p", [CMID, 1], F32, isOutput=False)
    mom3 = nc.declare_dram_parameter("mom3", [CMID, CMID + 1], F32, isOutput=True)
    n_real_chunks = (NSLAB + 127) // 128          # 254
    last_k = NSLAB - (n_real_chunks - 1) * 128    # 116
    with tile.TileContext(nc) as tc:
        with tc.tile_pool(name="csb", bufs=1) as csb, \
             tc.tile_pool(name="isb", bufs=3) as isb, \
             tc.tile_pool(name="hsb", bufs=3) as hsb, \
             tc.tile_pool(name="tps", bufs=4, space="PSUM") as tps, \
             tc.tile_pool(name="mps", bufs=1, space="PSUM") as mps, \
             tc.tile_pool(name="rsb", bufs=4) as rsb, \
             tc.tile_pool(name="osb", bufs=1) as osb:
            idt = csb.tile([CMID, CMID], F32, tag="idt")
            make_identity(nc, idt[:])
            b2t = csb.tile([CMID, 1], F32, tag="b2t")
            nc.sync.dma_start(out=b2t[:], in_=b2p[:])
            acc = mps.tile([CMID, CMID + 1], F32)
            i = 0
            n_mm = n_real_chunks * repeat
            for d in [dd for _ in range(repeat) for dd in range(NDT)]:
                if d * DTS >= NSLAB:
                    continue
                ot = isb.tile([128, DTS], FR, tag="ot")
                nc.sync.dma_start(out=ot[:], in_=oft[:, d * DTS:(d + 1) * DTS])
                h2 = hsb.tile([CMID, DTS], F32, tag="h2")
                act_in = ot[0:CMID, :]
                if FR == mybir.dt.float32r:
                    act_in = act_in.bitcast(F32)
                nc.scalar.activation(out=h2[:], in_=act_in,
                                     func=RELU, bias=b2t[:], scale=1.0)
                for grp in range(2):  # groups of 4 x 128-site chunks
                    base = d * (DTS // 128) + grp * 4
                    if base >= n_real_chunks:
                        break
                    ng = min(4, n_real_chunks - base)
                    tp = tps.tile([128, 4, CMID], F32, tag="tp")  # 1 bank
                    for a in range(ng):
                        sub = grp * 4 + a
                        nc.tensor.transpose(
                            out=tp[:, a, :], in_=h2[:, sub * 128:(sub + 1) * 128],
                            identity=idt[:])
                    rt = rsb.tile([128, 4, CMID + 1], F32, tag="rt")
                    nc.vector.tensor_copy(out=rt[:, 0:ng, 0:CMID], in_=tp[:, 0:ng, :])
                    nc.vector.memset(rt[:, 0:ng, CMID:CMID + 1], 1.0)
                    for a in range(ng):
                        chunk = base + a
                        k = 128 if chunk < n_real_chunks - 1 else last_k
                        nc.tensor.matmul(out=acc[:], lhsT=rt[0:k, a, 0:CMID],
                                         rhs=rt[0:k, a, 0:CMID + 1],
                                         start=(i == 0), stop=(i == n_mm - 1))
                        i += 1
            res = osb.tile([CMID, CMID + 1], F32)
            nc.scalar.copy(out=res[:], in_=acc[:])
            nc.sync.dma_start(out=mom3[:], in_=res[:])
    nc.compile()
    return nc


# ----------------------------------------------------- L4: output projections
def build_l4(repeat=1):
    nc = bacc.Bacc()
    oft = nc.declare_dram_parameter("oft", [128, NPAD], FR, isOutput=False)
    wwa = nc.declare_dram_parameter("wwa", [128, 128], FR, isOutput=False)
    wwb = nc.declare_dram_parameter("wwb", [128, 128], FR, isOutput=False)
    b2p = nc.declare_dram_parameter("b2p", [CMID, 1], F32, isOutput=False)
    bsa = nc.declare_dram_parameter("bsa", [128, 1], F32, isOutput=False)
    bsb = nc.declare_dram_parameter("bsb", [128, 1], F32, isOutput=False)
    outt = nc.declare_dram_parameter("outt", [COUT, NPAD], F32, isOutput=True)
    with tile.TileContext(nc) as tc:
        with tc.tile_pool(name="csb", bufs=1) as csb, \
             tc.tile_pool(name="isb", bufs=4) as isb, \
             tc.tile_pool(name="yps", bufs=2, space="PSUM") as yps, \
             tc.tile_pool(name="osb", bufs=3) as osb:
            wwa_t = csb.tile([128, 128], FR, tag="wwa")
            nc.sync.dma_start(out=wwa_t[:], in_=wwa[:])
            wwb_t = csb.tile([128, 128], FR, tag="wwb")
            nc.sync.dma_start(out=wwb_t[:], in_=wwb[:])
            b2t = csb.tile([CMID, 1], F32, tag="b2t")
            nc.sync.dma_start(out=b2t[:], in_=b2p[:])
            bsa_t = csb.tile([128, 1], F32, tag="bsa")
            nc.sync.dma_start(out=bsa_t[:], in_=bsa[:])
            bsb_t = csb.tile([128, 1], F32, tag="bsb")
            nc.sync.dma_start(out=bsb_t[:], in_=bsb[:])
            for d in [dd for _ in range(repeat) for dd in range(NDT)]:
                sl = slice(d * DTS, (d + 1) * DTS)
                ot = isb.tile([128, DTS], FR, tag="ot")
                nc.sync.dma_start(out=ot[:], in_=oft[:, sl])
                # in-place: top half <- relu(out2 + b2hat)
                act_in = ot[0:CMID, :]
                if FR == mybir.dt.float32r:
                    act_in = act_in.bitcast(F32)
                nc.scalar.activation(out=ot[0:CMID, :], in_=act_in,
                                     func=RELU, bias=b2t[:], scale=1.0)
                oa = osb.tile([128, DTS], F32, tag="oa")
                ob = osb.tile([128, DTS], F32, tag="ob")
                for sub in range(DTS // TS):
                    s2_ = slice(sub * TS, (sub + 1) * TS)
                    ya = yps.tile([128, TS], F32, tag=f"ya{sub}")
                    yb = yps.tile([128, TS], F32, tag=f"yb{sub}")
                    nc.tensor.matmul(out=ya[:], lhsT=wwa_t[:], rhs=ot[:, s2_],
                                     start=True, stop=True)
                    nc.tensor.matmul(out=yb[:], lhsT=wwb_t[:], rhs=ot[:, s2_],
                                     start=True, stop=True)
                    if sub == 0:
                        nc.vector.tensor_scalar(
                            out=oa[:, s2_], in0=ya[:], scalar1=bsa_t[:],
                            scalar2=0.0, op0=mybir.AluOpType.add,
                            op1=mybir.AluOpType.max)
                        nc.scalar.activation(out=ob[:, s2_], in_=yb[:], func=RELU,
                                             bias=bsb_t[:], scale=1.0)
                    else:
                        nc.scalar.activation(out=oa[:, s2_], in_=ya[:], func=RELU,
                                             bias=bsa_t[:], scale=1.0)
                        nc.vector.tensor_scalar(
                            out=ob[:, s2_], in0=yb[:], scalar1=bsb_t[:],
                            scalar2=0.0, op0=mybir.AluOpType.add,
                            op1=mybir.AluOpType.max)
                nc.sync.dma_start(out=outt[0:128, sl], in_=oa[:])
                nc.sync.dma_start(out=outt[128:256, sl], in_=ob[:])
    nc.compile()
    return nc


def _get(name, builder):
    if name not in _BUILT:
        _BUILT[name] = builder()
    return _BUILT[name]


# ---------------------------------------------------------------- host driver
def kernel(features, nbr_idx, W1, g1, b1, Wk, g2, b2, W3, g3, b3, Ws, gs, bs):
    features = np.asarray(features, dtype=np.float32)
    nbr_idx = np.asarray(nbr_idx, dtype=np.int32)
    W1 = np.asarray(W1, dtype=np.float32)
    g1 = np.asarray(g1, dtype=np.float32); b1 = np.asarray(b1, dtype=np.float32)
    Wk = np.asarray(Wk, dtype=np.float32)
    g2 = np.asarray(g2, dtype=np.float32); b2 = np.asarray(b2, dtype=np.float32)
    W3 = np.asarray(W3, dtype=np.float32)
    g3 = np.asarray(g3, dtype=np.float32); b3 = np.asarray(b3, dtype=np.float32)
    Ws = np.asarray(Ws, dtype=np.float32)
    gs = np.asarray(gs, dtype=np.float32); bs = np.asarray(bs, dtype=np.float32)

    # ---- L1: feature moments per core
    nc1 = _get("l1", build_l1)
    l1_maps = []
    feat_slabs = []
    for c in range(CORES):
        slab = np.zeros((NPAD, CIN), np.float32)
        slab[:NSLAB] = features[c * NSLAB:(c + 1) * NSLAB]
        feat_slabs.append(slab)
        l1_maps.append({"feat": slab})
    r1 = _run("l1", nc1, l1_maps)
    mom = np.zeros((CIN, CIN), np.float64)
    for c in range(CORES):
        m_ = r1[c]["mom"].astype(np.float64)
        mom += m_[:CIN] + m_[CIN:]
    M = mom / N
    mu = features.astype(np.float64).sum(axis=0) / N

    def bn_from_moments(W, g, b):
        m = mu @ W
        e2 = ((M @ W) * W).sum(axis=0)
        v = np.maximum(e2 - m * m, 0.0)
        a = g.astype(np.float64) / np.sqrt(v + BN_EPS)
        bb = b.astype(np.float64) - m * a
        return a, bb

    a1, be1 = bn_from_moments(W1, g1, b1)
    as_, bes = bn_from_moments(Ws, gs, bs)

    # fold BN1 scale into W1; synthetic x* row with relu(x*@W1p + be1) == 0
    W1p = W1.astype(np.float64) * a1[None, :]
    zstar = -MARGIN - be1
    xstar = np.linalg.solve(W1p.T, zstar)
    chk = (xstar @ W1p) + be1
    assert chk.max() < -MARGIN * 0.5, f"x* margin violated: {chk.max()}"
    xstar = xstar.astype(np.float32)

    # ---- build per-core gathered feature blocks (the halo gather, on host)
    featpad = np.vstack([features, xstar[None, :]])        # row N = x*
    idx_all = np.where(nbr_idx >= 0, nbr_idx, N)            # [N, 9]
    nc2 = _get("l2", build_l2)
    wbd = np.zeros((128, 128), np.float32)
    wbd[:64, :64] = W1p
    wbd[64:, 64:] = W1p
    wkp = np.zeros((NBLK, 128, CMID), np.float32)
    for bpair in range(4):
        wkp[bpair, :64] = Wk[2 * bpair]
        wkp[bpair, 64:] = Wk[2 * bpair + 1]
    wkp[4, :64] = Wk[8]
    b1p = np.tile(be1.astype(np.float32), 2)[:, None]
    l2_maps = []
    for c in range(CORES):
        idx = np.full((NPAD, K9), N, np.int32)
        idx[:NSLAB] = idx_all[c * NSLAB:(c + 1) * NSLAB]
        g = featpad[idx]                                    # [NPAD, 9, 64]
        gf = np.zeros((NBLK, 128, NPAD), L2DT_NP)
        for bpair in range(4):
            gf[bpair, :64] = g[:, 2 * bpair, :].T
            gf[bpair, 64:] = g[:, 2 * bpair + 1, :].T
        gf[4, :64] = g[:, 8, :].T
        l2_maps.append({"gf": gf, "wbd": wbd.astype(L2DT_NP),
                        "wkp": wkp.astype(L2DT_NP), "b1p": b1p})
    r2 = _run("l2", nc2, l2_maps)

    # bn2 stats on host from the (quantized) conv output itself -- consistent
    # with what L3/L4 consume. Padded sites are exact zeros (x* trick).
    ssum = np.zeros(CMID, np.float64)
    sqsum = np.zeros(CMID, np.float64)
    o2t_fulls = []
    for c in range(CORES):
        dev = r2[c]["o2t"]                       # [128, NPAD//2] packed
        o2t_full = np.ascontiguousarray(
            dev.reshape(2, CMID, NDT, TS).transpose(1, 2, 0, 3)
        ).reshape(CMID, NPAD)
        o2t_fulls.append(o2t_full)
        v = o2t_full.astype(np.float64)
        ssum += v.sum(axis=1)
        sqsum += (v * v).sum(axis=1)
    mean2 = ssum / N
    var2 = np.maximum(sqsum / N - mean2 * mean2, 0.0)
    a2 = g2.astype(np.float64) / np.sqrt(var2 + BN_EPS)
    be2 = b2.astype(np.float64) - mean2 * a2
    assert (a2 > 0).all(), "BN2 scale must be positive for relu folding"
    b2hat = (be2 / a2).astype(np.float32)[:, None]   # hhat = relu(out2 + b2hat)

    # ---- L3: hhat moments  (h2 = a2 * hhat; a2 accounted in W3-side math)
    nc3 = _get("l3", build_l3)
    ofts = []
    l3_maps = []
    for c in range(CORES):
        oft = np.zeros((128, NPAD), FR_NP)
        oft[:CMID] = o2t_fulls[c].astype(FR_NP)
        oft[CMID:] = feat_slabs[c].T.astype(FR_NP)
        ofts.append(oft)
        l3_maps.append({"oft": oft, "b2p": b2hat})
    r3 = _run("l3", nc3, l3_maps)
    mom3 = np.zeros((CMID, CMID + 1), np.float64)
    for c in range(CORES):
        mom3 += r3[c]["mom3"].astype(np.float64)
    M3 = mom3[:, :CMID] / N          # moments of hhat
    mu3 = mom3[:, CMID] / N
    W3t = W3.astype(np.float64) * a2[:, None]        # h2 @ W3 == hhat @ W3t
    m3 = mu3 @ W3t
    e23 = ((M3 @ W3t) * W3t).sum(axis=0)
    v3 = np.maximum(e23 - m3 * m3, 0.0)
    a3 = g3.astype(np.float64) / np.sqrt(v3 + BN_EPS)
    be3 = b3.astype(np.float64) - m3 * a3

    # ---- L4: final projections, BN folded into weights
    nc4 = _get("l4", build_l4)
    W3pp = (W3t * a3[None, :]).astype(np.float32)    # rows: hhat channels
    Wsp = (Ws.astype(np.float64) * as_[None, :]).astype(np.float32)
    bsum = (be3 + bes).astype(np.float32)
    wwa = np.vstack([W3pp[:, :128], Wsp[:, :128]])
    wwb = np.vstack([W3pp[:, 128:], Wsp[:, 128:]])
    bsa = bsum[:128, None].copy()
    bsb = bsum[128:, None].copy()
    l4_maps = []
    for c in range(CORES):
        l4_maps.append({"oft": ofts[c], "wwa": wwa.astype(FR_NP),
                        "wwb": wwb.astype(FR_NP),
                        "b2p": b2hat, "bsa": bsa, "bsb": bsb})
    r4 = _run("l4", nc4, l4_maps)

    out = np.empty((N, COUT), np.float32)
    for c in range(CORES):
        out[c * NSLAB:(c + 1) * NSLAB] = r4[c]["outt"][:, :NSLAB].T
    return out


# revision 15
# speedup vs baseline: 4.1635x; 4.1635x over previous
"""Trainium2 Bass kernel for nn_BottleneckSparse2D (submanifold sparse bottleneck
block, gnn_message_passing).

Strategy (8 NeuronCores, SPMD, sites sharded):
  N=260000 active sites are sharded as contiguous slabs of 32500 sites/core,
  zero-padded to 32768. The rulebook gather (halo gather) is applied on the
  host to the *input features* (gather commutes with the 1x1 conv + BN + relu),
  so each core receives dense, GEMM-ready, transposed per-offset feature
  blocks. BN batch statistics are reduced across cores on the host between
  launches (sums / second-moment matrices only; tiny tensors).

  L1: per-core feature moments  [sum(x), x^T x]      (for BN1 + BN_s stats)
  L2: z = g_k @ W1' -> relu(z+b1) -> sum_k Wk^T h_k  (the 3x3 subm conv),
      emits out2_raw^T slab + BN2 partial sums. BN1 scale is folded into W1.
  L3: hhat = relu(out2_raw + b2/a2) -> moments       (for BN3; a2 handled on host)
  L4: out^T = relu(W3''^T hhat^T + Ws'^T feat^T + beta)

  Invalid rulebook entries (and padded sites) gather a synthetic feature row
  x* chosen so that relu(x*@W1' + b1) == 0 exactly (with margin), so they
  contribute nothing to the conv and keep padded sites' outputs at exact 0
  (which keeps the cross-core BN2 sums clean).

L2 matmuls run in bf16 by default (BASS_L2_DT=float32r to fall back);
L3/L4 matmuls in float32r. Stats are accumulated in fp32/PSUM throughout.
"""

import os
import numpy as np
import ml_dtypes

import concourse.bacc as bacc
import concourse.tile as tile
from concourse import bass, mybir
from concourse.bass_utils import run_bass_kernel_spmd
from concourse.masks import make_identity

F32 = mybir.dt.float32
L2DT = getattr(mybir.dt, os.environ.get("BASS_L2_DT", "bfloat16"))
FR = getattr(mybir.dt, os.environ.get("BASS_L34_DT", "bfloat16"))
FR_NP = None  # set below
L2DT_NP = mybir.dt.np(L2DT)
FR_NP = mybir.dt.np(FR)

N = 260000
CORES = 8
NSLAB = N // CORES            # 32500
NPAD = 32768                  # per-core padded slab
TS = 512                      # PE site-tile
DTS = 1024                    # DMA site-tile
NDT = NPAD // DTS             # 32
CIN = 64
CMID = 64
COUT = 256
K9 = 9
NBLK = 5                      # 4 offset-pairs + 1 solo block
BN_EPS = 1e-5
MARGIN = 4.0

TRACE = bool(int(os.environ.get("BASS_KERNEL_TRACE", "0")))
LAST_EXEC_NS = {}
LAST_IN_MAPS = {}

_BUILT = {}

RELU = mybir.ActivationFunctionType.Relu


def _run(name, nc, in_maps):
    if TRACE:
        LAST_IN_MAPS[name] = in_maps
    res = run_bass_kernel_spmd(nc, in_maps, core_ids=list(range(CORES)))
    LAST_EXEC_NS[name] = res.exec_time_ns
    return res.results


# ---------------------------------------------------------------- L1: moments
def build_l1(repeat=1):
    nc = bacc.Bacc()
    feat = nc.declare_dram_parameter("feat", [NPAD, CIN], F32, isOutput=False)
    mom = nc.declare_dram_parameter("mom", [128, CIN], F32, isOutput=True)
    with tile.TileContext(nc) as tc:
        with tc.tile_pool(name="sb", bufs=2) as sb, \
             tc.tile_pool(name="ps", bufs=1, space="PSUM") as ps, \
             tc.tile_pool(name="osb", bufs=1) as osb:
            acc0 = ps.tile([128, CIN], F32, tag="acc0")  # col-group 0 (rows 0:64)
            acc1 = ps.tile([128, CIN], F32, tag="acc1")  # col-group 1 (rows 64:128)
            # contiguous per-partition layout: partition p holds sites
            # [p*256, (p+1)*256); chunk j covers t in [j*64, (j+1)*64)
            feat_r = feat[:].rearrange("(p t) c -> p t c", p=128)  # [128, 256, 64]
            nchunk = 4
            tper = 256 // nchunk
            n_mm = nchunk * tper * repeat
            i = 0
            for j in [jj for _ in range(repeat) for jj in range(nchunk)]:
                ck = sb.tile([128, tper, CIN], F32, tag="ck")
                nc.sync.dma_start(out=ck[:], in_=feat_r[:, j * tper:(j + 1) * tper, :])
                for t in range(tper):
                    half = t % 2
                    acc = acc0 if half == 0 else acc1
                    nc.tensor.matmul(
                        out=acc[half * CIN:(half + 1) * CIN, :],
                        lhsT=ck[:, t, :],
                        rhs=ck[:, t, :],
                        tile_position=(0, half * CIN),
                        start=(i <= 1), stop=(i >= n_mm - 2),
                    )
                    i += 1
            res = osb.tile([128, CIN], F32)
            nc.scalar.copy(out=res[0:CIN, :], in_=acc0[0:CIN, :])
            nc.scalar.copy(out=res[CIN:128, :], in_=acc1[CIN:128, :])
            nc.sync.dma_start(out=mom[:], in_=res[:])
    nc.compile()
    return nc


# ------------------------------------------------------- L2: submanifold conv
def build_l2(repeat=1):
    nc = bacc.Bacc()
    gf = nc.declare_dram_parameter("gf", [NBLK, 128, NPAD], L2DT, isOutput=False)
    wbd = nc.declare_dram_parameter("wbd", [128, 128], L2DT, isOutput=False)
    wkp = nc.declare_dram_parameter("wkp", [NBLK, 128, CMID], L2DT, isOutput=False)
    b1p = nc.declare_dram_parameter("b1p", [128, 1], F32, isOutput=False)
    o2t = nc.declare_dram_parameter("o2t", [128, NPAD // 2], L2DT, isOutput=True)
    with tile.TileContext(nc) as tc:
        with tc.tile_pool(name="wsb", bufs=1) as wsb, \
             tc.tile_pool(name="gsb", bufs=3) as gsb, \
             tc.tile_pool(name="hsb", bufs=5) as hsb, \
             tc.tile_pool(name="zps", bufs=3, space="PSUM") as zps, \
             tc.tile_pool(name="ops", bufs=2, space="PSUM") as ops, \
             tc.tile_pool(name="osb", bufs=4) as osb:
            wbd_t = wsb.tile([128, 128], L2DT, tag="wbd")
            nc.sync.dma_start(out=wbd_t[:], in_=wbd[:])
            wkp_t = wsb.tile([128, NBLK, CMID], L2DT, tag="wkp")
            nc.sync.dma_start(out=wkp_t[:], in_=wkp[:].rearrange("b p c -> p b c"))
            b1t = wsb.tile([128, 1], F32, tag="b1t")
            nc.sync.dma_start(out=b1t[:], in_=b1p[:])
            for d in [dd for _ in range(repeat) for dd in range(NDT)]:
                sl = slice(d * DTS, (d + 1) * DTS)
                # out2 for this 1024-site tile: partitions 0:64 = sites
                # [d*1024, +512) ; partitions 64:128 = sites [d*1024+512, +512)
                o = ops.tile([128, TS], F32, tag="o")  # 1 bank
                for b in range(NBLK):
                    gt = gsb.tile([128, DTS], L2DT, tag=f"g{b}")
                    nc.sync.dma_start(out=gt[:], in_=gf[b, :, sl])
                    z = zps.tile([128, DTS], F32, tag="z")  # 2 banks
                    nc.tensor.matmul(out=z[:, 0:TS], lhsT=wbd_t[:],
                                     rhs=gt[:, 0:TS], start=True, stop=True)
                    nc.tensor.matmul(out=z[:, TS:DTS], lhsT=wbd_t[:],
                                     rhs=gt[:, TS:DTS], start=True, stop=True)
                    h = hsb.tile([128, DTS], L2DT, tag="h")
                    if b >= 3:
                        nc.vector.tensor_scalar(
                            out=h[:], in0=z[:], scalar1=b1t[:], scalar2=0.0,
                            op0=mybir.AluOpType.add, op1=mybir.AluOpType.max)
                    else:
                        # relu(z + b1) on ACT (scale pre-folded into W1)
                        nc.scalar.activation(out=h[:], in_=z[:], func=RELU,
                                             bias=b1t[:], scale=1.0)
                    # two concurrent M=64 matmuls in PE column groups 0/1
                    nc.tensor.matmul(out=o[0:CMID, :], lhsT=wkp_t[:, b, :],
                                     rhs=h[:, 0:TS], tile_position=(0, 0),
                                     start=(b == 0), stop=(b == NBLK - 1))
                    nc.tensor.matmul(out=o[CMID:128, :], lhsT=wkp_t[:, b, :],
                                     rhs=h[:, TS:DTS], tile_position=(0, 64),
                                     start=(b == 0), stop=(b == NBLK - 1))
                ot = osb.tile([128, TS], L2DT, tag="ot")
                nc.vector.tensor_copy(out=ot[:], in_=o[:])
                nc.sync.dma_start(out=o2t[:, d * TS:(d + 1) * TS], in_=ot[:])
    nc.compile()
    return nc


# ------------------------------------------------------------- L3: h2 moments
def build_l3(repeat=1):
    nc = bacc.Bacc()
    oft = nc.declare_dram_parameter("oft", [128, NPAD], FR, isOutput=False)
    b2p = nc.declare_dram_parameter("b2p", [CMID, 1], F32, isOutput=False)
    mom3 = nc.declare_dram_parameter("mom3", [CMID, CMID + 1], F32, isOutput=True)
    n_real_chunks = (NSLAB + 127) // 128          # 254
    last_k = NSLAB - (n_real_chunks - 1) * 128    # 116
    with tile.TileContext(nc) as tc:
        with tc.tile_pool(name="csb", bufs=1) as csb, \
             tc.tile_pool(name="isb", bufs=4) as isb, \
             tc.tile_pool(name="hsb", bufs=4) as hsb, \
             tc.tile_pool(name="tps", bufs=6, space="PSUM") as tps, \
             tc.tile_pool(name="mps", bufs=1, space="PSUM") as mps, \
             tc.tile_pool(name="rsb", bufs=8) as rsb, \
             tc.tile_pool(name="ssb", bufs=8) as ssb, \
             tc.tile_pool(name="osb", bufs=1) as osb:
            idt = csb.tile([CMID, CMID], F32, tag="idt")
            make_identity(nc, idt[:])
            b2t = csb.tile([CMID, 1], F32, tag="b2t")
            nc.sync.dma_start(out=b2t[:], in_=b2p[:])
            hsum = csb.tile([CMID, 1], F32, tag="hsum")
            nc.vector.memset(hsum[:], 0.0)
            acc = mps.tile([CMID, CMID], F32)
            i = 0
            n_mm = n_real_chunks * repeat
            for d in [dd for _ in range(repeat) for dd in range(NDT)]:
                ot = isb.tile([128, DTS], FR, tag="ot")
                nc.sync.dma_start(out=ot[:], in_=oft[:, d * DTS:(d + 1) * DTS])
                h2 = hsb.tile([CMID, DTS], F32, tag="h2")
                act_in = ot[0:CMID, :]
                if FR == mybir.dt.float32r:
                    act_in = act_in.bitcast(F32)
                hp = ssb.tile([CMID, 1], F32, tag="hp")
                nc.scalar.activation(out=h2[:], in_=act_in,
                                     func=RELU, bias=b2t[:], scale=1.0,
                                     accum_out=hp[:])
                nc.vector.tensor_add(out=hsum[:], in0=hsum[:], in1=hp[:])
                for grp in range(2):  # groups of 4 x 128-site chunks
                    base = d * (DTS // 128) + grp * 4
                    if base >= n_real_chunks:
                        break
                    ng = min(4, n_real_chunks - base)
                    tp = tps.tile([128, 4, CMID], F32, tag="tp")  # 1 bank
                    for a in range(ng):
                        sub = grp * 4 + a
                        nc.tensor.transpose(
                            out=tp[:, a, :], in_=h2[:, sub * 128:(sub + 1) * 128],
                            identity=idt[:])
                    rt = rsb.tile([128, 4, CMID], F32, tag="rt")
                    nc.vector.tensor_copy(out=rt[:, 0:ng, :], in_=tp[:, 0:ng, :])
                    for a in range(ng):
                        chunk = base + a
                        k = 128 if chunk < n_real_chunks - 1 else last_k
                        nc.tensor.matmul(out=acc[:], lhsT=rt[0:k, a, :],
                                         rhs=rt[0:k, a, :],
                                         start=(i == 0), stop=(i == n_mm - 1))
                        i += 1
            res = osb.tile([CMID, CMID + 1], F32)
            nc.scalar.copy(out=res[:, 0:CMID], in_=acc[:])
            nc.vector.tensor_copy(out=res[:, CMID:CMID + 1], in_=hsum[:])
            nc.sync.dma_start(out=mom3[:], in_=res[:])
    nc.compile()
    return nc


# ----------------------------------------------------- L4: output projections
def build_l4(repeat=1):
    nc = bacc.Bacc()
    oft = nc.declare_dram_parameter("oft", [128, NPAD], FR, isOutput=False)
    wwa = nc.declare_dram_parameter("wwa", [128, 128], FR, isOutput=False)
    wwb = nc.declare_dram_parameter("wwb", [128, 128], FR, isOutput=False)
    b2p = nc.declare_dram_parameter("b2p", [CMID, 1], F32, isOutput=False)
    bsa = nc.declare_dram_parameter("bsa", [128, 1], F32, isOutput=False)
    bsb = nc.declare_dram_parameter("bsb", [128, 1], F32, isOutput=False)
    outt = nc.declare_dram_parameter("outt", [COUT, NPAD], F32, isOutput=True)
    with tile.TileContext(nc) as tc:
        with tc.tile_pool(name="csb", bufs=1) as csb, \
             tc.tile_pool(name="isb", bufs=4) as isb, \
             tc.tile_pool(name="yps", bufs=2, space="PSUM") as yps, \
             tc.tile_pool(name="osb", bufs=3) as osb:
            wwa_t = csb.tile([128, 128], FR, tag="wwa")
            nc.sync.dma_start(out=wwa_t[:], in_=wwa[:])
            wwb_t = csb.tile([128, 128], FR, tag="wwb")
            nc.sync.dma_start(out=wwb_t[:], in_=wwb[:])
            b2t = csb.tile([CMID, 1], F32, tag="b2t")
            nc.sync.dma_start(out=b2t[:], in_=b2p[:])
            bsa_t = csb.tile([128, 1], F32, tag="bsa")
            nc.sync.dma_start(out=bsa_t[:], in_=bsa[:])
            bsb_t = csb.tile([128, 1], F32, tag="bsb")
            nc.sync.dma_start(out=bsb_t[:], in_=bsb[:])
            for d in [dd for _ in range(repeat) for dd in range(NDT)]:
                sl = slice(d * DTS, (d + 1) * DTS)
                ot = isb.tile([128, DTS], FR, tag="ot")
                nc.sync.dma_start(out=ot[:], in_=oft[:, sl])
                # in-place: top half <- relu(out2 + b2hat)
                act_in = ot[0:CMID, :]
                if FR == mybir.dt.float32r:
                    act_in = act_in.bitcast(F32)
                nc.scalar.activation(out=ot[0:CMID, :], in_=act_in,
                                     func=RELU, bias=b2t[:], scale=1.0)
                oa = osb.tile([128, DTS], F32, tag="oa")
                ob = osb.tile([128, DTS], F32, tag="ob")
                for sub in range(DTS // TS):
                    s2_ = slice(sub * TS, (sub + 1) * TS)
                    ya = yps.tile([128, TS], F32, tag=f"ya{sub}")
                    yb = yps.tile([128, TS], F32, tag=f"yb{sub}")
                    nc.tensor.matmul(out=ya[:], lhsT=wwa_t[:], rhs=ot[:, s2_],
                                     start=True, stop=True)
                    nc.tensor.matmul(out=yb[:], lhsT=wwb_t[:], rhs=ot[:, s2_],
                                     start=True, stop=True)
                    if sub == 0:
                        nc.vector.tensor_scalar(
                            out=oa[:, s2_], in0=ya[:], scalar1=bsa_t[:],
                            scalar2=0.0, op0=mybir.AluOpType.add,
                            op1=mybir.AluOpType.max)
                        nc.scalar.activation(out=ob[:, s2_], in_=yb[:], func=RELU,
                                             bias=bsb_t[:], scale=1.0)
                    else:
                        nc.scalar.activation(out=oa[:, s2_], in_=ya[:], func=RELU,
                                             bias=bsa_t[:], scale=1.0)
                        nc.vector.tensor_scalar(
                            out=ob[:, s2_], in0=yb[:], scalar1=bsb_t[:],
                            scalar2=0.0, op0=mybir.AluOpType.add,
                            op1=mybir.AluOpType.max)
                nc.sync.dma_start(out=outt[0:128, sl], in_=oa[:])
                nc.sync.dma_start(out=outt[128:256, sl], in_=ob[:])
    nc.compile()
    return nc


def _get(name, builder):
    if name not in _BUILT:
        _BUILT[name] = builder()
    return _BUILT[name]


# ---------------------------------------------------------------- host driver
def kernel(features, nbr_idx, W1, g1, b1, Wk, g2, b2, W3, g3, b3, Ws, gs, bs):
    features = np.asarray(features, dtype=np.float32)
    nbr_idx = np.asarray(nbr_idx, dtype=np.int32)
    W1 = np.asarray(W1, dtype=np.float32)
    g1 = np.asarray(g1, dtype=np.float32); b1 = np.asarray(b1, dtype=np.float32)
    Wk = np.asarray(Wk, dtype=np.float32)
    g2 = np.asarray(g2, dtype=np.float32); b2 = np.asarray(b2, dtype=np.float32)
    W3 = np.asarray(W3, dtype=np.float32)
    g3 = np.asarray(g3, dtype=np.float32); b3 = np.asarray(b3, dtype=np.float32)
    Ws = np.asarray(Ws, dtype=np.float32)
    gs = np.asarray(gs, dtype=np.float32); bs = np.asarray(bs, dtype=np.float32)

    # ---- L1: feature moments per core
    nc1 = _get("l1", build_l1)
    l1_maps = []
    feat_slabs = []
    for c in range(CORES):
        slab = np.zeros((NPAD, CIN), np.float32)
        slab[:NSLAB] = features[c * NSLAB:(c + 1) * NSLAB]
        feat_slabs.append(slab)
        l1_maps.append({"feat": slab})
    r1 = _run("l1", nc1, l1_maps)
    mom = np.zeros((CIN, CIN), np.float64)
    for c in range(CORES):
        m_ = r1[c]["mom"].astype(np.float64)
        mom += m_[:CIN] + m_[CIN:]
    M = mom / N
    mu = features.astype(np.float64).sum(axis=0) / N

    def bn_from_moments(W, g, b):
        m = mu @ W
        e2 = ((M @ W) * W).sum(axis=0)
        v = np.maximum(e2 - m * m, 0.0)
        a = g.astype(np.float64) / np.sqrt(v + BN_EPS)
        bb = b.astype(np.float64) - m * a
        return a, bb

    a1, be1 = bn_from_moments(W1, g1, b1)
    as_, bes = bn_from_moments(Ws, gs, bs)

    # fold BN1 scale into W1; synthetic x* row with relu(x*@W1p + be1) == 0
    W1p = W1.astype(np.float64) * a1[None, :]
    zstar = -MARGIN - be1
    xstar = np.linalg.solve(W1p.T, zstar)
    chk = (xstar @ W1p) + be1
    assert chk.max() < -MARGIN * 0.5, f"x* margin violated: {chk.max()}"
    xstar = xstar.astype(np.float32)

    # ---- build per-core gathered feature blocks (the halo gather, on host)
    featpad = np.vstack([features, xstar[None, :]])        # row N = x*
    idx_all = np.where(nbr_idx >= 0, nbr_idx, N)            # [N, 9]
    nc2 = _get("l2", build_l2)
    wbd = np.zeros((128, 128), np.float32)
    wbd[:64, :64] = W1p
    wbd[64:, 64:] = W1p
    wkp = np.zeros((NBLK, 128, CMID), np.float32)
    for bpair in range(4):
        wkp[bpair, :64] = Wk[2 * bpair]
        wkp[bpair, 64:] = Wk[2 * bpair + 1]
    wkp[4, :64] = Wk[8]
    b1p = np.tile(be1.astype(np.float32), 2)[:, None]
    l2_maps = []
    for c in range(CORES):
        idx = np.full((NPAD, K9), N, np.int32)
        idx[:NSLAB] = idx_all[c * NSLAB:(c + 1) * NSLAB]
        g = featpad[idx]                                    # [NPAD, 9, 64]
        gf = np.zeros((NBLK, 128, NPAD), L2DT_NP)
        for bpair in range(4):
            gf[bpair, :64] = g[:, 2 * bpair, :].T
            gf[bpair, 64:] = g[:, 2 * bpair + 1, :].T
        gf[4, :64] = g[:, 8, :].T
        l2_maps.append({"gf": gf, "wbd": wbd.astype(L2DT_NP),
                        "wkp": wkp.astype(L2DT_NP), "b1p": b1p})
    r2 = _run("l2", nc2, l2_maps)

    # bn2 stats on host from the (quantized) conv output itself -- consistent
    # with what L3/L4 consume. Padded sites are exact zeros (x* trick).
    ssum = np.zeros(CMID, np.float64)
    sqsum = np.zeros(CMID, np.float64)
    o2t_fulls = []
    for c in range(CORES):
        dev = r2[c]["o2t"]                       # [128, NPAD//2] packed
        o2t_full = np.ascontiguousarray(
            dev.reshape(2, CMID, NDT, TS).transpose(1, 2, 0, 3)
        ).reshape(CMID, NPAD)
        o2t_fulls.append(o2t_full)
        v = o2t_full.astype(np.float64)
        ssum += v.sum(axis=1)
        sqsum += (v * v).sum(axis=1)
    mean2 = ssum / N
    var2 = np.maximum(sqsum / N - mean2 * mean2, 0.0)
    a2 = g2.astype(np.float64) / np.sqrt(var2 + BN_EPS)
    be2 = b2.astype(np.float64) - mean2 * a2
    assert (a2 > 0).all(), "BN2 scale must be positive for relu folding"
    b2hat = (be2 / a2).astype(np.float32)[:, None]   # hhat = relu(out2 + b2hat)

    # ---- L3: hhat moments  (h2 = a2 * hhat; a2 accounted in W3-side math)
    nc3 = _get("l3", build_l3)
    ofts = []
    l3_maps = []
    for c in range(CORES):
        oft = np.zeros((128, NPAD), FR_NP)
        oft[:CMID] = o2t_fulls[c].astype(FR_NP)
        oft[CMID:] = feat_slabs[c].T.astype(FR_NP)
        ofts.append(oft)
        l3_maps.append({"oft": oft, "b2p": b2hat})
    r3 = _run("l3", nc3, l3_maps)
    mom3 = np.zeros((CMID, CMID + 1), np.float64)
    for c in range(CORES):
        mom3 += r3[c]["mom3"].astype(np.float64)
    # device hsum includes the (NPAD - NSLAB) padded sites, whose hhat is
    # exactly relu(b2hat); subtract them.
    pad_h = np.maximum(b2hat[:, 0].astype(np.float64), 0.0)
    M3 = mom3[:, :CMID] / N          # moments of hhat (pads are excluded by k)
    mu3 = (mom3[:, CMID] - CORES * (NPAD - NSLAB) * pad_h) / N
    W3t = W3.astype(np.float64) * a2[:, None]        # h2 @ W3 == hhat @ W3t
    m3 = mu3 @ W3t
    e23 = ((M3 @ W3t) * W3t).sum(axis=0)
    v3 = np.maximum(e23 - m3 * m3, 0.0)
    a3 = g3.astype(np.float64) / np.sqrt(v3 + BN_EPS)
    be3 = b3.astype(np.float64) - m3 * a3

    # ---- L4: final projections, BN folded into weights
    nc4 = _get("l4", build_l4)
    W3pp = (W3t * a3[None, :]).astype(np.float32)    # rows: hhat channels
    Wsp = (Ws.astype(np.float64) * as_[None, :]).astype(np.float32)
    bsum = (be3 + bes).astype(np.float32)
    wwa = np.vstack([W3pp[:, :128], Wsp[:, :128]])
    wwb = np.vstack([W3pp[:, 128:], Wsp[:, 128:]])
    bsa = bsum[:128, None].copy()
    bsb = bsum[128:, None].copy()
    l4_maps = []
    for c in range(CORES):
        l4_maps.append({"oft": ofts[c], "wwa": wwa.astype(FR_NP),
                        "wwb": wwb.astype(FR_NP),
                        "b2p": b2hat, "bsa": bsa, "bsb": bsb})
    r4 = _run("l4", nc4, l4_maps)

    out = np.empty((N, COUT), np.float32)
    for c in range(CORES):
        out[c * NSLAB:(c + 1) * NSLAB] = r4[c]["outt"][:, :NSLAB].T
    return out
